# revision 1
# baseline (speedup 1.0000x reference)
"""CTRNN (Dale-constrained leaky RNN) Trainium2 kernel.

Math (per reference):
    Weff    = |Wahh| * mask
    xin_t   = x_t @ Wahx.T + bah
    ah_{t+1} = 0.9*ah_t + 0.1*(retanh(ah_t) @ Weff.T + xin_t)
    hs[t]   = retanh(ah_{t+1});   y = hs @ Wyh.T + by
    retanh(a) = max(tanh(a), 0) = tanh(max(a, 0))

Strategy: data-parallel over batch (B=64 -> 8 batches per NeuronCore); the
sequential T=1000 scan runs locally per core in neuron-major layout
[128 part, chunk, batch], with the recurrent weight as the PE stationary
operand (bf16, fp32 PSUM accumulate) so no per-step transposes are needed.
The input drive is precomputed on-device into HBM; the readout y is folded
into the scan as one small matmul block every U steps from an SBUF h-ring.
"""

import numpy as np
import ml_dtypes

import concourse.bass as bass
import concourse.bacc as bacc
import concourse.mybir as mybir
from concourse.tile import TileContext
from concourse.bass_utils import run_bass_kernel_spmd
from concourse.alu_op_type import AluOpType

F32 = mybir.dt.float32
BF16 = mybir.dt.bfloat16
AF = mybir.ActivationFunctionType

B, T, NI, N, NO = 64, 1000, 128, 1024, 64
NCORES = 8
BL = B // NCORES            # batches per core
MC = N // 128               # m-chunks (output neuron chunks)
KC = N // 128               # k-chunks (contraction chunks)
U = 50                      # timesteps per block
NB = T // U                 # blocks
DT = 0.1                    # dt/tau
DECAY = 1.0 - DT

TRACE = False               # set by test harness for profiling
LAST_RESULTS = None         # BassKernelResults of the last run


def _bcast_ap(t, shape_counts, steps):
    """Build an AP on tile t with explicit [step, count] pairs (after the
    partition dim, which is taken from t)."""
    ap = [t.ap[0]] + [[s, c] for s, c in zip(steps, shape_counts)]
    return bass.AP(tensor=t.tensor, offset=t.offset, ap=ap)


def _build_nc():
    nc = bacc.Bacc("TRN2", target_bir_lowering=False)

    x_d = nc.dram_tensor("x", [BL * T, NI], BF16, kind="ExternalInput")
    wq_d = nc.dram_tensor("wq", [128, KC, MC, 128], BF16, kind="ExternalInput")
    wx_d = nc.dram_tensor("wx", [NI, MC, 128], BF16, kind="ExternalInput")
    wy_d = nc.dram_tensor("wy", [128, KC, NO], BF16, kind="ExternalInput")
    bah_d = nc.dram_tensor("bah", [128, MC], F32, kind="ExternalInput")
    ah0_d = nc.dram_tensor("ah0", [128, MC], F32, kind="ExternalInput")
    by_d = nc.dram_tensor("by", [NO, 1], F32, kind="ExternalInput")
    y_d = nc.dram_tensor("y", [NO, T, BL], F32, kind="ExternalOutput")
    xin_d = nc.dram_tensor("xin", [NB, 128, MC, BL, U], F32, kind="Internal")

    with TileContext(nc) as tc:
        with tc.tile_pool(name="consts", bufs=1) as consts:
            wsta = consts.tile([128, KC, MC, 128], BF16)
            nc.sync.dma_start(wsta, wq_d[:])
            wx = consts.tile([NI, MC, 128], BF16)
            nc.sync.dma_start(wx, wx_d[:])
            wy = consts.tile([128, KC, NO], BF16)
            nc.sync.dma_start(wy, wy_d[:])
            bahT = consts.tile([128, MC], F32)
            nc.sync.dma_start(bahT, bah_d[:])
            ah0T = consts.tile([128, MC], F32)
            nc.sync.dma_start(ah0T, ah0_d[:])
            byv = consts.tile([NO, 1], F32)
            nc.sync.dma_start(byv, by_d[:])

            xT = consts.tile([NI, BL * T], BF16)        # x transposed, bf16
            yb = consts.tile([NO, NB, U, BL], F32)      # y accumulator
            ah = consts.tile([128, MC, BL], F32)        # recurrent state
            ring = consts.tile([128, U + 1, MC, BL], BF16)  # h ring (slot, chunk, b)

            # ---- Phase 1: transpose x -> xT via DMA xbar transpose ----
            nc.sync.dma_start_transpose(xT, x_d[:])

            # ---- Phase 2: xin = 0.1*(x @ Wahx.T + bah), blocked to HBM ----
            xTv = xT.rearrange("p (b t) -> p b t", t=T)
            with tc.tile_pool(name="ph2", bufs=3) as ph2, \
                 tc.tile_pool(name="ph2ps", bufs=2, space="PSUM") as ph2ps:
                for j in range(NB):
                    for mi in range(MC):
                        px = ph2ps.tile([128, BL, U], F32, tag="px")
                        nc.tensor.matmul(px, lhsT=wx[:, mi, :],
                                         rhs=xTv[:, :, j * U:(j + 1) * U],
                                         start=True, stop=True)
                        xs = ph2.tile([128, BL, U], F32, tag="xs")
                        nc.scalar.activation(xs, px, AF.Identity,
                                             bias=bahT[:, mi:mi + 1], scale=1.0)
                        nc.sync.dma_start(xin_d[j, :, mi, :, :], xs)

            # ---- Init: ah = broadcast(ah0), ring[0] = retanh(ah) ----
            with tc.tile_pool(name="initp", bufs=1) as initp:
                ah0b = _bcast_ap(ah0T, [MC, BL], [1, 0])
                nc.vector.tensor_copy(ah, ah0b)
                r0t = initp.tile([128, MC, BL], F32)
                nc.vector.tensor_scalar_max(r0t, ah, 0.0)
                nc.scalar.activation(ring[:, 0], r0t, AF.Tanh)

            # ---- Phase 3: the scan ----
            H = MC // 2
            with tc.tile_pool(name="scan", bufs=2) as scan_p, \
                 tc.tile_pool(name="scps", bufs=2, space="PSUM") as scps, \
                 tc.tile_pool(name="yps", bufs=2, space="PSUM") as yps:
                with tc.For_i(0, NB, 1,
                              hint_engines=(mybir.EngineType.PE,)) as j:
                    xin_blk = scan_p.tile([128, MC, BL, U], F32, tag="xinb")
                    nc.sync.dma_start(xin_blk, xin_d[bass.ds(j, 1), :, :, :, :])
                    for th in range(U):
                        s_r = th
                        s_w = th + 1
                        u_t = scan_p.tile([128, MC, BL], F32, tag="u")
                        nc.vector.scalar_tensor_tensor(
                            out=u_t, in0=ah, scalar=DECAY,
                            in1=xin_blk[:, :, :, th],
                            op0=AluOpType.mult, op1=AluOpType.add)
                        for half in range(2):
                            ps = scps.tile([128, H, BL], F32,
                                           tag=f"ps{half}")
                            for mloc in range(H):
                                mi = half * H + mloc
                                for ki in range(KC):
                                    nc.tensor.matmul(
                                        ps[:, mloc, :],
                                        lhsT=wsta[:, ki, mi, :],
                                        rhs=ring[:, s_r, ki, :],
                                        start=(ki == 0), stop=(ki == KC - 1))
                            sl = slice(half * H, half * H + H)
                            nc.vector.tensor_tensor(
                                out=ah[:, sl, :], in0=ps, in1=u_t[:, sl, :],
                                op=AluOpType.add)
                            rr = scan_p.tile([128, H, BL], F32, tag="rr")
                            nc.vector.tensor_scalar_max(rr, ah[:, sl, :], 0.0)
                            nc.scalar.activation(ring[:, s_w, sl, :], rr,
                                                 AF.Tanh)
                    # carry last h into slot 0 for the next block
                    nc.vector.tensor_copy(ring[:, 0], ring[:, U])
                    # y block: y[o, t, b] over this block's 50 steps
                    yp = yps.tile([NO, U, BL], F32, tag="yp")
                    for ki in range(KC):
                        nc.tensor.matmul(yp, lhsT=wy[:, ki, :],
                                         rhs=ring[:, 1:U + 1, ki, :],
                                         start=(ki == 0), stop=(ki == KC - 1))
                    ybv = yb.rearrange("o n u b -> o n (u b)")
                    ypv = yp.rearrange("o u b -> o (u b)")
                    nc.vector.tensor_copy(
                        ybv[:, bass.ds(j, 1), :],
                        _bcast_ap(ypv, [1, U * BL], [0, 1]))

            # ---- Post: add by, write y out ----
            ybf = yb.rearrange("o n u b -> o (n u b)")
            nc.scalar.activation(ybf, ybf, AF.Identity, bias=byv[:, 0:1],
                                 scale=1.0)
            nc.sync.dma_start(y_d[:], yb.rearrange("o n u b -> o (n u) b"))

    nc.compile()
    return nc


_NC_CACHE = {}


def _get_nc():
    if "nc" not in _NC_CACHE:
        _NC_CACHE["nc"] = _build_nc()
    return _NC_CACHE["nc"]


def prepare_in_maps(x, Wahx, Wahh, Wyh, bah, by, ah0, mask):
    bf16 = ml_dtypes.bfloat16
    x = np.asarray(x, np.float32)
    Wahx = np.asarray(Wahx, np.float32)
    Wahh = np.asarray(Wahh, np.float32)
    Wyh = np.asarray(Wyh, np.float32)
    bah = np.asarray(bah, np.float32)
    by = np.asarray(by, np.float32)
    ah0 = np.asarray(ah0, np.float32)
    mask = np.asarray(mask, np.float32)

    weff = np.abs(Wahh) * mask                       # [m, k]
    wq = (DT * weff).reshape(MC, 128, KC, 128)       # [mi, mm, ki, kk]
    wq_l = np.ascontiguousarray(wq.transpose(3, 2, 0, 1)).astype(bf16)
    wx_l = np.ascontiguousarray(
        (DT * Wahx).T.reshape(NI, MC, 128)).astype(bf16)
    wy_l = np.ascontiguousarray(
        Wyh.T.reshape(KC, 128, NO).transpose(1, 0, 2)).astype(bf16)
    bah_l = np.ascontiguousarray((DT * bah).reshape(MC, 128).T,
                                 dtype=np.float32)
    ah0_l = np.ascontiguousarray(ah0.reshape(MC, 128).T, dtype=np.float32)
    by_l = np.ascontiguousarray(by.reshape(NO, 1), dtype=np.float32)

    x16 = x.reshape(B, T * NI).astype(bf16)
    in_maps = []
    for c in range(NCORES):
        xc = np.ascontiguousarray(
            x16[c * BL:(c + 1) * BL].reshape(BL * T, NI))
        in_maps.append(dict(x=xc, wq=wq_l, wx=wx_l, wy=wy_l, bah=bah_l,
                            ah0=ah0_l, by=by_l))
    return in_maps


def kernel(x, Wahx, Wahh, Wyh, bah, by, ah0, mask):
    global LAST_RESULTS
    in_maps = prepare_in_maps(x, Wahx, Wahh, Wyh, bah, by, ah0, mask)
    nc = _get_nc()
    res = run_bass_kernel_spmd(nc, in_maps, core_ids=list(range(NCORES)),
                               trace=TRACE)
    LAST_RESULTS = res

    out = np.empty((B, T, NO), np.float32)
    for c in range(NCORES):
        yc = np.asarray(res.results[c]["y"], np.float32)   # [NO, T, BL]
        out[c * BL:(c + 1) * BL] = yc.transpose(2, 1, 0)
    return out



# revision 6
# speedup vs baseline: 1.1259x; 1.1259x over previous
"""CTRNN (Dale-constrained leaky RNN) Trainium2 kernel, v2.

Math (per reference):
    Weff    = |Wahh| * mask
    xin_t   = x_t @ Wahx.T + bah
    ah_{t+1} = 0.9*ah_t + 0.1*(retanh(ah_t) @ Weff.T + xin_t)
    hs[t]   = retanh(ah_{t+1});   y = hs @ Wyh.T + by
    retanh(a) = max(tanh(a), 0) = tanh(max(a, 0))

Strategy: data-parallel over batch (B=64 -> 8 per NeuronCore), neuron-major
state layout [128, chunk, batch], recurrent weight stationary on the PE
(bf16, fp32 PSUM accumulate).

v2 scan structure (vs v1): the per-step matmuls are split into two k-waves:
wave A (k-chunks 0..3) only reads the first half of h_t, wave B (k-chunks
4..7) the second half. Each of the 8 m-chunk accumulation groups lives in
its own PSUM bank so all groups stay open across the waves. The state
update runs per m-half as soon as its groups close, so wave A of step t+1
(gated only on the half-0 tanh) issues while the half-1 update of step t is
still in flight -- the PE never idles long enough for the HAM clock gate to
re-throttle it to 1.2 GHz (which is where v1 spent the whole scan).
"""

import numpy as np
import ml_dtypes

import concourse.bass as bass
import concourse.bacc as bacc
import concourse.mybir as mybir
from concourse.tile import TileContext
from concourse.bass_utils import run_bass_kernel_spmd
from concourse.alu_op_type import AluOpType

F32 = mybir.dt.float32
BF16 = mybir.dt.bfloat16
AF = mybir.ActivationFunctionType

B, T, NI, N, NO = 64, 1000, 128, 1024, 64
NCORES = 8
BL = B // NCORES            # batches per core
MC = N // 128               # m-chunks (output neuron chunks)
KC = N // 128               # k-chunks (contraction chunks)
U = 50                      # timesteps per block
NB = T // U                 # blocks
DT = 0.1                    # dt/tau
DECAY = 1.0 - DT
KA = 4                      # k-chunks in wave A (rest in wave B)
MH = MC // 2                # m-chunks per update half

TRACE = False               # set by test harness for profiling
LAST_RESULTS = None         # BassKernelResults of the last run


def _bcast_ap(t, shape_counts, steps):
    """Build an AP on tile t with explicit [step, count] pairs (after the
    partition dim, which is taken from t)."""
    ap = [t.ap[0]] + [[s, c] for s, c in zip(steps, shape_counts)]
    return bass.AP(tensor=t.tensor, offset=t.offset, ap=ap)


def _order_dep(after, before):
    """Order-only scheduling edge: `after` must be queued after `before`."""
    from concourse.tile_rust import add_dep_helper
    a = getattr(after, "ins", after)
    b = getattr(before, "ins", before)
    add_dep_helper(a, b, reason="manual queue order")


def _build_nc():
    nc = bacc.Bacc("TRN2", target_bir_lowering=False)

    x_d = nc.dram_tensor("x", [BL * T, NI], BF16, kind="ExternalInput")
    wq_d = nc.dram_tensor("wq", [128, KC, MC, 128], BF16, kind="ExternalInput")
    wx_d = nc.dram_tensor("wx", [NI, MC, 128], BF16, kind="ExternalInput")
    wy_d = nc.dram_tensor("wy", [128, KC, NO], BF16, kind="ExternalInput")
    bah_d = nc.dram_tensor("bah", [128, MC], F32, kind="ExternalInput")
    ah0_d = nc.dram_tensor("ah0", [128, MC], F32, kind="ExternalInput")
    by_d = nc.dram_tensor("by", [NO, 1], F32, kind="ExternalInput")
    y_d = nc.dram_tensor("y", [NO, T, BL], F32, kind="ExternalOutput")
    xin_d = nc.dram_tensor("xin", [NB, 128, MC, BL, U], F32, kind="Internal")

    with TileContext(nc) as tc:
        with tc.tile_pool(name="consts", bufs=1) as consts, \
             tc.tile_pool(name="bigps", bufs=1, space="PSUM") as bigps:
            wsta = consts.tile([128, KC, MC, 128], BF16)
            nc.sync.dma_start(wsta, wq_d[:])
            wx = consts.tile([NI, MC, 128], BF16)
            nc.sync.dma_start(wx, wx_d[:])
            wy = consts.tile([128, KC, NO], BF16)
            nc.sync.dma_start(wy, wy_d[:])
            bahT = consts.tile([128, MC], F32)
            nc.sync.dma_start(bahT, bah_d[:])
            ah0T = consts.tile([128, MC], F32)
            nc.sync.dma_start(ah0T, ah0_d[:])
            byv = consts.tile([NO, 1], F32)
            nc.sync.dma_start(byv, by_d[:])

            xT = consts.tile([NI, BL * T], BF16)        # x transposed, bf16
            yb = consts.tile([NO, NB, U, BL], F32)      # y accumulator
            ah = consts.tile([128, MC, BL], F32)        # recurrent state
            ring = consts.tile([128, U, KC, BL], BF16)  # h ring (slot, chunk, b)

            # One PSUM tensor spanning all 8 banks: scan accumulation group
            # for m-chunk mi lives at big[:, mi, 0:BL] (its own 2KB bank /
            # zero-region, so all 8 groups can be open at once); the y
            # readout aliases the tail of bank 7; phase-2 xin tiles rotate
            # through banks. All usage windows are disjoint in time.
            big = bigps.tile([128, MC, 512], F32)

            # ---- Phase 1: transpose x -> xT via DMA xbar transpose ----
            nc.sync.dma_start_transpose(xT, x_d[:])

            # ---- Phase 2: xin = 0.1*(x @ Wahx.T + bah), blocked to HBM ----
            xTv = xT.rearrange("p (b t) -> p b t", t=T)
            with tc.tile_pool(name="ph2", bufs=3) as ph2:
                for j in range(NB):
                    for mi in range(MC):
                        px = big[:, mi, 0:BL * U]
                        nc.tensor.matmul(px, lhsT=wx[:, mi, :],
                                         rhs=xTv[:, :, j * U:(j + 1) * U],
                                         start=True, stop=True)
                        xs = ph2.tile([128, BL, U], F32, tag="xs")
                        nc.scalar.activation(
                            xs, px.rearrange("p (b u) -> p b u", u=U),
                            AF.Identity,
                            bias=bahT[:, mi:mi + 1], scale=1.0)
                        nc.sync.dma_start(xin_d[j, :, mi, :, :], xs)

            # ---- Init: ah = broadcast(ah0), ring[U-1] = retanh(ah) ----
            with tc.tile_pool(name="initp", bufs=1) as initp:
                ah0b = _bcast_ap(ah0T, [MC, BL], [1, 0])
                nc.vector.tensor_copy(ah, ah0b)
                r0t = initp.tile([128, MC, BL], F32)
                nc.vector.tensor_scalar_max(r0t, ah, 0.0)
                nc.scalar.activation(ring[:, U - 1], r0t, AF.Tanh)

            # ---- Phase 3: the scan ----
            with tc.tile_pool(name="scan", bufs=2) as scan_p, \
                 tc.tile_pool(name="upool", bufs=2) as upool, \
                 tc.tile_pool(name="rrpool", bufs=4) as rrp:
                with tc.For_i(0, NB, 1, hint_engines=(mybir.EngineType.PE,),
                              staggered_reset=True) as j:
                    xin_blk = scan_p.tile([128, MC, BL, U], F32, tag="xinb")
                    nc.sync.dma_start(xin_blk, xin_d[bass.ds(j, 1), :, :, :, :])
                    xin_v = xin_blk  # [128, MC, BL, U]

                    u_cur = upool.tile([128, MC, BL], F32, tag="u")
                    nc.vector.scalar_tensor_tensor(
                        out=u_cur, in0=ah, scalar=DECAY,
                        in1=xin_v[:, :, :, 0],
                        op0=AluOpType.mult, op1=AluOpType.add)

                    for th in range(U):
                        s_r = (th - 1) % U
                        s_w = th
                        # wave A: k-chunks 0..KA-1 for every m-chunk
                        for mi in range(MC):
                            for ki in range(KA):
                                nc.tensor.matmul(
                                    big[:, mi, 0:BL],
                                    lhsT=wsta[:, ki, mi, :],
                                    rhs=ring[:, s_r, ki, :],
                                    start=(ki == 0), stop=False)
                        prev_ts = None
                        tanh_insts = []
                        for half in range(2):
                            # wave B for this m-half: k-chunks KA..KC-1
                            for mloc in range(MH):
                                mi = half * MH + mloc
                                for ki in range(KA, KC):
                                    nc.tensor.matmul(
                                        big[:, mi, 0:BL],
                                        lhsT=wsta[:, ki, mi, :],
                                        rhs=ring[:, s_r, ki, :],
                                        start=False, stop=(ki == KC - 1))
                            sl = slice(half * MH, half * MH + MH)
                            ps_view = big[:, sl, 0:BL]
                            tt = nc.vector.tensor_tensor(
                                out=ah[:, sl, :], in0=ps_view,
                                in1=u_cur[:, sl, :], op=AluOpType.add)
                            if prev_ts is not None:
                                _order_dep(tt, prev_ts)
                            rr = rrp.tile([128, MH, BL], F32, tag="rr")
                            ts = nc.vector.tensor_scalar_max(rr, ah[:, sl, :],
                                                             0.0)
                            prev_ts = ts
                            tk = nc.scalar.activation(ring[:, s_w, sl, :], rr,
                                                      AF.Tanh)
                            if tanh_insts:
                                _order_dep(tk, tanh_insts[-1])
                            tanh_insts.append(tk)
                        if th < U - 1:
                            u_nxt = upool.tile([128, MC, BL], F32, tag="u")
                            stt = nc.vector.scalar_tensor_tensor(
                                out=u_nxt, in0=ah, scalar=DECAY,
                                in1=xin_v[:, :, :, th + 1],
                                op0=AluOpType.mult, op1=AluOpType.add)
                            _order_dep(stt, prev_ts)
                            u_cur = u_nxt

                    # y block: y[o, t, b] over this block's U steps
                    ypv = big[0:NO, MC - 1, 64:64 + U * BL]
                    for ki in range(KC):
                        nc.tensor.matmul(ypv, lhsT=wy[:, ki, :],
                                         rhs=ring[:, :, ki, :],
                                         start=(ki == 0), stop=(ki == KC - 1))
                    ybv = yb.rearrange("o n u b -> o n (u b)")
                    nc.vector.tensor_copy(
                        ybv[:, bass.ds(j, 1), :],
                        _bcast_ap(ypv, [1, U * BL], [0, 1]))

            # ---- Post: add by, write y out ----
            ybf = yb.rearrange("o n u b -> o (n u b)")
            nc.scalar.activation(ybf, ybf, AF.Identity, bias=byv[:, 0:1],
                                 scale=1.0)
            nc.sync.dma_start(y_d[:], yb.rearrange("o n u b -> o (n u) b"))

    nc.compile()
    return nc


_NC_CACHE = {}


def _get_nc():
    if "nc" not in _NC_CACHE:
        _NC_CACHE["nc"] = _build_nc()
    return _NC_CACHE["nc"]


def prepare_in_maps(x, Wahx, Wahh, Wyh, bah, by, ah0, mask):
    bf16 = ml_dtypes.bfloat16
    x = np.asarray(x, np.float32)
    Wahx = np.asarray(Wahx, np.float32)
    Wahh = np.asarray(Wahh, np.float32)
    Wyh = np.asarray(Wyh, np.float32)
    bah = np.asarray(bah, np.float32)
    by = np.asarray(by, np.float32)
    ah0 = np.asarray(ah0, np.float32)
    mask = np.asarray(mask, np.float32)

    weff = np.abs(Wahh) * mask                       # [m, k]
    wq = (DT * weff).reshape(MC, 128, KC, 128)       # [mi, mm, ki, kk]
    wq_l = np.ascontiguousarray(wq.transpose(3, 2, 0, 1)).astype(bf16)
    wx_l = np.ascontiguousarray(
        (DT * Wahx).T.reshape(NI, MC, 128)).astype(bf16)
    wy_l = np.ascontiguousarray(
        Wyh.T.reshape(KC, 128, NO).transpose(1, 0, 2)).astype(bf16)
    bah_l = np.ascontiguousarray((DT * bah).reshape(MC, 128).T,
                                 dtype=np.float32)
    ah0_l = np.ascontiguousarray(ah0.reshape(MC, 128).T, dtype=np.float32)
    by_l = np.ascontiguousarray(by.reshape(NO, 1), dtype=np.float32)

    x16 = x.reshape(B, T * NI).astype(bf16)
    in_maps = []
    for c in range(NCORES):
        xc = np.ascontiguousarray(
            x16[c * BL:(c + 1) * BL].reshape(BL * T, NI))
        in_maps.append(dict(x=xc, wq=wq_l, wx=wx_l, wy=wy_l, bah=bah_l,
                            ah0=ah0_l, by=by_l))
    return in_maps


def kernel(x, Wahx, Wahh, Wyh, bah, by, ah0, mask):
    global LAST_RESULTS
    in_maps = prepare_in_maps(x, Wahx, Wahh, Wyh, bah, by, ah0, mask)
    nc = _get_nc()
    res = run_bass_kernel_spmd(nc, in_maps, core_ids=list(range(NCORES)),
                               trace=TRACE)
    LAST_RESULTS = res

    out = np.empty((B, T, NO), np.float32)
    for c in range(NCORES):
        yc = np.asarray(res.results[c]["y"], np.float32)   # [NO, T, BL]
        out[c * BL:(c + 1) * BL] = yc.transpose(2, 1, 0)
    return out


# revision 7
# speedup vs baseline: 1.1640x; 1.0338x over previous
"""CTRNN (Dale-constrained leaky RNN) Trainium2 kernel, v3.

Math (per reference):
    Weff    = |Wahh| * mask
    xin_t   = x_t @ Wahx.T + bah
    ah_{t+1} = 0.9*ah_t + 0.1*(retanh(ah_t) @ Weff.T + xin_t)
    hs[t]   = retanh(ah_{t+1});   y = hs @ Wyh.T + by
    retanh(a) = max(tanh(a), 0) = tanh(max(a, 0))

Strategy: data-parallel over batch (B=64 -> 8 per NeuronCore), neuron-major
state [128, chunk, batch], recurrent weight stationary on the PE (bf16,
fp32 PSUM accumulate). The per-step PE stream is LDWEIGHTS-bandwidth bound
(~26 ns per 128x128 weight tile, clock-independent), so the step period is
set by the dependency cycle, not PE throughput.

v3 structure:
- The input drive AND bias are accumulated directly into PSUM: per block,
  bank mi is opened with a rank-1 bias matmul (0.1*bah chunk x ones) plus
  an input matmul (0.1*Wahx chunk @ x_t), and the 50 steps' recurrent
  matmuls accumulate on top at element offsets t*BL. The state update is a
  single fused DVE op: ah = 0.9*ah + psum. No xin HBM round trip, no
  per-block DMA, no separate u op.
- Parity-alternating skewed schedule: at step t the halves H[t%2]/H[1-t%2]
  update first/second. Quad order (first, k-early)(second, k-early)
  (first, k-late -> update first)(second, k-late -> update second) makes
  the binding dependency cycle one 16-matmul quad + the update chain,
  instead of 48 matmuls + chain.
- The y readout runs per block out of the PSUM bank tails (offsets
  400..512 of banks 4..7), overlapped at block boundaries.
"""

import numpy as np
import ml_dtypes

import concourse.bass as bass
import concourse.bacc as bacc
import concourse.mybir as mybir
from concourse.tile import TileContext
from concourse.bass_utils import run_bass_kernel_spmd
from concourse.alu_op_type import AluOpType

F32 = mybir.dt.float32
BF16 = mybir.dt.bfloat16
AF = mybir.ActivationFunctionType

B, T, NI, N, NO = 64, 1000, 128, 1024, 64
NCORES = 8
BL = B // NCORES            # batches per core
MC = N // 128               # m-chunks (output neuron chunks)
KC = N // 128               # k-chunks (contraction chunks)
U = 50                      # timesteps per block
NB = T // U                 # blocks
DT = 0.1                    # dt/tau
DECAY = 1.0 - DT
MH = MC // 2                # m-chunks per half
KH = KC // 2                # k-chunks per half
# y readout slot ranges per PSUM bank tail (112 fp32 capacity each)
YSPLIT = [(0, 14), (14, 28), (28, 42), (42, 50)]

TRACE = False               # set by test harness for profiling
LAST_RESULTS = None         # BassKernelResults of the last run


def _bcast_ap(t, shape_counts, steps):
    """Build an AP on tile t with explicit [step, count] pairs (after the
    partition dim, which is taken from t)."""
    ap = [t.ap[0]] + [[s, c] for s, c in zip(steps, shape_counts)]
    return bass.AP(tensor=t.tensor, offset=t.offset, ap=ap)


def _order_dep(after, before):
    """Order-only scheduling edge: `after` must be queued after `before`."""
    from concourse.tile_rust import add_dep_helper
    a = getattr(after, "ins", after)
    b = getattr(before, "ins", before)
    add_dep_helper(a, b, reason="manual queue order")


def _build_nc():
    nc = bacc.Bacc("TRN2", target_bir_lowering=False)

    x_d = nc.dram_tensor("x", [BL * T, NI], BF16, kind="ExternalInput")
    wq_d = nc.dram_tensor("wq", [128, KC, MC, 128], BF16, kind="ExternalInput")
    wx_d = nc.dram_tensor("wx", [NI, MC, 128], BF16, kind="ExternalInput")
    wy_d = nc.dram_tensor("wy", [128, KC, NO], BF16, kind="ExternalInput")
    bahq_d = nc.dram_tensor("bahq", [1, MC, 128], BF16, kind="ExternalInput")
    ones_d = nc.dram_tensor("ones", [1, U * BL], BF16, kind="ExternalInput")
    ah0_d = nc.dram_tensor("ah0", [128, MC], F32, kind="ExternalInput")
    by_d = nc.dram_tensor("by", [NO, 1], F32, kind="ExternalInput")
    y_d = nc.dram_tensor("y", [NO, T, BL], F32, kind="ExternalOutput")

    with TileContext(nc) as tc:
        with tc.tile_pool(name="consts", bufs=1) as consts, \
             tc.tile_pool(name="bigps", bufs=1, space="PSUM") as bigps:
            wsta = consts.tile([128, KC, MC, 128], BF16)
            nc.sync.dma_start(wsta, wq_d[:])
            wx = consts.tile([NI, MC, 128], BF16)
            nc.sync.dma_start(wx, wx_d[:])
            wy = consts.tile([128, KC, NO], BF16)
            nc.sync.dma_start(wy, wy_d[:])
            bahq = consts.tile([1, MC, 128], BF16)
            nc.sync.dma_start(bahq, bahq_d[:])
            ones = consts.tile([1, U * BL], BF16)
            nc.sync.dma_start(ones, ones_d[:])
            ah0T = consts.tile([128, MC], F32)
            nc.sync.dma_start(ah0T, ah0_d[:])
            byv = consts.tile([NO, 1], F32)
            nc.sync.dma_start(byv, by_d[:])

            xT = consts.tile([NI, BL * T], BF16)        # x transposed, bf16
            yb = consts.tile([NO, NB, U, BL], F32)      # y accumulator
            ah = consts.tile([128, MC, BL], F32)        # recurrent state
            ring = consts.tile([128, U, KC, BL], BF16)  # h ring (slot, chunk, b)

            big = bigps.tile([128, MC, 512], F32)       # all 8 PSUM banks

            # ---- Phase 1: transpose x -> xT via DMA xbar transpose ----
            nc.sync.dma_start_transpose(xT, x_d[:])
            # x as [p, block, u, b] for the per-block xin matmuls
            xTv = xT.rearrange("p (b nb u) -> p nb u b", nb=NB, u=U)

            # ---- Init: ah = broadcast(ah0), ring[U-1] = retanh(ah) ----
            with tc.tile_pool(name="initp", bufs=1) as initp:
                ah0b = _bcast_ap(ah0T, [MC, BL], [1, 0])
                nc.vector.tensor_copy(ah, ah0b)
                r0t = initp.tile([128, MC, BL], F32)
                nc.vector.tensor_scalar_max(r0t, ah, 0.0)
                nc.scalar.activation(ring[:, U - 1], r0t, AF.Tanh)

            # ---- The scan ----
            with tc.tile_pool(name="rrpool", bufs=4) as rrp:
                with tc.For_i(0, NB, 1, hint_engines=(mybir.EngineType.PE,),
                              staggered_reset=True) as j:
                    # open each bank's block group: bias + input drive
                    for mi in range(MC):
                        pxv = big[:, mi, 0:U * BL]
                        nc.tensor.matmul(pxv, lhsT=bahq[0:1, mi, :],
                                         rhs=ones[0:1, :],
                                         start=True, stop=False)
                        nc.tensor.matmul(pxv, lhsT=wx[:, mi, :],
                                         rhs=xTv[:, bass.ds(j, 1), :, :],
                                         start=False, stop=False,
                                         skip_group_check=True)

                    for th in range(U):
                        hf = th % 2          # first-updated half
                        hs = 1 - hf
                        s_r = (th - 1) % U
                        s_w = th

                        def quad(mh, kh, stop):
                            for mloc in range(MH):
                                mi = mh * MH + mloc
                                for kloc in range(KH):
                                    ki = kh * KH + kloc
                                    nc.tensor.matmul(
                                        big[:, mi, th * BL:(th + 1) * BL],
                                        lhsT=wsta[:, ki, mi, :],
                                        rhs=ring[:, s_r, ki, :],
                                        start=False,
                                        stop=(stop and ki == KC - 1),
                                        skip_group_check=True)

                        def update(mh, prev_ts, prev_tanh):
                            sl = slice(mh * MH, mh * MH + MH)
                            psv = big[:, sl, th * BL:(th + 1) * BL]
                            stt = nc.vector.scalar_tensor_tensor(
                                out=ah[:, sl, :], in0=ah[:, sl, :],
                                scalar=DECAY, in1=psv,
                                op0=AluOpType.mult, op1=AluOpType.add)
                            if prev_ts is not None:
                                _order_dep(stt, prev_ts)
                            rr = rrp.tile([128, MH, BL], F32, tag="rr")
                            ts = nc.vector.tensor_scalar_max(
                                rr, ah[:, sl, :], 0.0)
                            tk = nc.scalar.activation(ring[:, s_w, sl, :], rr,
                                                      AF.Tanh)
                            if prev_tanh is not None:
                                _order_dep(tk, prev_tanh)
                            return ts, tk

                        last = (th == U - 1)
                        quad(hf, hs, False)            # q1: first m, early k
                        quad(hs, hs, False)            # q2: second m, early k
                        quad(hf, hf, last)             # q3: closes first m
                        ts1, tk1 = update(hf, None, None)
                        quad(hs, hf, last)             # q4: closes second m
                        update(hs, ts1, tk1)

                    # y readout from the PSUM bank tails (banks 4..7)
                    copies = []
                    for q, (a, b) in enumerate(YSPLIT):
                        fd = (b - a) * BL
                        ypv = big[0:NO, MH + q, 400:400 + fd]
                        for ki in range(KC):
                            nc.tensor.matmul(ypv, lhsT=wy[:, ki, :],
                                             rhs=ring[:, a:b, ki, :],
                                             start=(ki == 0),
                                             stop=(ki == KC - 1),
                                             skip_group_check=True)
                        copies.append((ypv, a, b, fd))
                    ybv = yb.rearrange("o n u b -> o n (u b)")
                    for ypv, a, b, fd in copies:
                        nc.vector.tensor_copy(
                            ybv[:, bass.ds(j, 1), a * BL:b * BL],
                            _bcast_ap(ypv, [1, fd], [0, 1]))

            # ---- Post: add by, write y out ----
            ybf = yb.rearrange("o n u b -> o (n u b)")
            nc.scalar.activation(ybf, ybf, AF.Identity, bias=byv[:, 0:1],
                                 scale=1.0)
            nc.sync.dma_start(y_d[:], yb.rearrange("o n u b -> o (n u) b"))

    nc.compile()
    return nc


_NC_CACHE = {}


def _get_nc():
    if "nc" not in _NC_CACHE:
        _NC_CACHE["nc"] = _build_nc()
    return _NC_CACHE["nc"]


def prepare_in_maps(x, Wahx, Wahh, Wyh, bah, by, ah0, mask):
    bf16 = ml_dtypes.bfloat16
    x = np.asarray(x, np.float32)
    Wahx = np.asarray(Wahx, np.float32)
    Wahh = np.asarray(Wahh, np.float32)
    Wyh = np.asarray(Wyh, np.float32)
    bah = np.asarray(bah, np.float32)
    by = np.asarray(by, np.float32)
    ah0 = np.asarray(ah0, np.float32)
    mask = np.asarray(mask, np.float32)

    weff = np.abs(Wahh) * mask                       # [m, k]
    wq = (DT * weff).reshape(MC, 128, KC, 128)       # [mi, mm, ki, kk]
    wq_l = np.ascontiguousarray(wq.transpose(3, 2, 0, 1)).astype(bf16)
    wx_l = np.ascontiguousarray(
        (DT * Wahx).T.reshape(NI, MC, 128)).astype(bf16)
    wy_l = np.ascontiguousarray(
        Wyh.T.reshape(KC, 128, NO).transpose(1, 0, 2)).astype(bf16)
    bahq_l = np.ascontiguousarray(
        (DT * bah).reshape(1, MC, 128)).astype(bf16)
    ones_l = np.ones((1, U * BL), dtype=bf16)
    ah0_l = np.ascontiguousarray(ah0.reshape(MC, 128).T, dtype=np.float32)
    by_l = np.ascontiguousarray(by.reshape(NO, 1), dtype=np.float32)

    x16 = x.reshape(B, T * NI).astype(bf16)
    in_maps = []
    for c in range(NCORES):
        xc = np.ascontiguousarray(
            x16[c * BL:(c + 1) * BL].reshape(BL * T, NI))
        in_maps.append(dict(x=xc, wq=wq_l, wx=wx_l, wy=wy_l, bahq=bahq_l,
                            ones=ones_l, ah0=ah0_l, by=by_l))
    return in_maps


def kernel(x, Wahx, Wahh, Wyh, bah, by, ah0, mask):
    global LAST_RESULTS
    in_maps = prepare_in_maps(x, Wahx, Wahh, Wyh, bah, by, ah0, mask)
    nc = _get_nc()
    res = run_bass_kernel_spmd(nc, in_maps, core_ids=list(range(NCORES)),
                               trace=TRACE)
    LAST_RESULTS = res

    out = np.empty((B, T, NO), np.float32)
    for c in range(NCORES):
        yc = np.asarray(res.results[c]["y"], np.float32)   # [NO, T, BL]
        out[c * BL:(c + 1) * BL] = yc.transpose(2, 1, 0)
    return out
